# revision 16
# baseline (speedup 1.0000x reference)
"""3-layer LSTM (B=256,T=512,I=256,H=512) + linear head on 8 NeuronCores.

Strategy: data-parallel over batch (32/core). Per step the gate matmul is
computed h-STATIONARY: the tiny h_t.T chunk ([128,32]) is the PE stationary
operand and the *weights* stream through the array as the moving operand,
split across the 4 PE column groups (tile_position=(0,32g)) so four N=512
weight streams run concurrently (~216ns per round of 2048 cols). This
removes the per-step LDWEIGHTS wall (8192 weight cols/step through the
slow load path) that bounds the weights-stationary form.

PSUM layout [32*slice+b, 128*q+jj] = gates.T: all 4 gates of a hidden unit
live on the same partition, so the sigmoid/tanh + cell update run with all
128 DVE/ACT lanes. h_t is transposed back to stationary form each step by
one [128,128] bf16 DMA-crossbar transpose (~1.2us, off the PE).

The 3 layers run as a wavefront (layer l computes step u-l in super-step u)
so the PE always has ~25 dense matmul rounds per super-step and the HAM
clock gate stays at 2.4GHz. h never leaves SBUF; biases enter via a K=1
ones-row matmul that also start=True-clears each psum accumulation group.
"""

import numpy as np
import ml_dtypes
from contextlib import ExitStack

import concourse.bass as bass
import concourse.bacc as bacc
import concourse.tile as tile
from concourse import mybir
from concourse.bass_utils import run_bass_kernel_spmd

BF16 = mybir.dt.bfloat16
F32 = mybir.dt.float32
AF = mybir.ActivationFunctionType

B, T, I, H, O = 256, 512, 256, 512, 3
NCORES = 8
BL = B // NCORES          # 32 batch rows per core
U = 255                   # super-steps per half-body (x chunk granularity)
NITER = 1                 # steady loop iterations; covers u in [2, 512)
SLACK = 4 * U * 32        # x prefetch overrun slack (cols)

# per-layer input K chunks (x for l0: 256 = 2 chunks; h for l1/l2: 4 chunks)
KIN = [2, 4, 4]


def _build():
    nc = bacc.Bacc("TRN2", target_bir_lowering=False, debug=False,
                   num_devices=NCORES)

    xt = nc.dram_tensor("x_t", (128, 2, T * BL + SLACK), BF16,
                        kind="ExternalInput")
    # packed weight streams: [input chunks | whh chunks], each chunk 2048 cols
    wpk = [nc.dram_tensor(f"w{l}", (128, (KIN[l] + 4) * 2048), BF16,
                          kind="ExternalInput") for l in range(3)]
    bias_d = [nc.dram_tensor(f"b{l}", (1, 2048), BF16, kind="ExternalInput")
              for l in range(3)]
    fcw_d = nc.dram_tensor("fcw", (128, 12), BF16, kind="ExternalInput")
    fcb_d = nc.dram_tensor("fcb", (3, 1), F32, kind="ExternalInput")
    out_d = nc.dram_tensor("out", (3, BL), F32, kind="ExternalOutput")

    with tile.TileContext(nc) as tc, ExitStack() as ctx:
        const = ctx.enter_context(tc.tile_pool(name="const", bufs=1))
        w_sb = [const.tile([128, (KIN[l] + 4) * 2048], BF16, tag=f"w{l}",
                           name=f"w_sb{l}") for l in range(3)]
        bias_sb = [const.tile([1, 2048], BF16, tag=f"b{l}", name=f"bias_sb{l}")
                   for l in range(3)]
        ones_sb = const.tile([1, BL], BF16, tag="ones")
        fcw_sb = const.tile([128, 12], BF16, tag="fcw")
        fcb_sb = const.tile([3, 1], F32, tag="fcb")
        xboot = const.tile([128, 2, 2 * BL], BF16, tag="xboot")
        for l in range(3):
            nc.sync.dma_start(w_sb[l][:], wpk[l].ap())
            nc.sync.dma_start(bias_sb[l][:], bias_d[l].ap())
        nc.sync.dma_start(fcw_sb[:], fcw_d.ap())
        nc.sync.dma_start(fcb_sb[:], fcb_d.ap())
        nc.sync.dma_start(xboot[:], xt.ap()[:, :, 0:2 * BL])
        nc.vector.memset(ones_sb[:], 1.0)

        st = ctx.enter_context(tc.tile_pool(name="st", bufs=1))
        # persistent state
        c_st = [st.tile([128, 128], F32, tag=f"c{l}", name=f"c_st{l}")
                for l in range(3)]
        hT = [[st.tile([128, 128], BF16, tag=f"hT{l}_{p}", name=f"hT{l}_{p}")
               for p in range(2)] for l in range(3)]

        def hT_ap(l, p, kc):
            return hT[l][p][:, 32 * kc:32 * kc + 32]
        h_sb = [[st.tile([128, 128], BF16, tag=f"h{l}_{p}", name=f"h_sb{l}_{p}")
                 for p in range(2)] for l in range(3)]
        gs = [[st.tile([128, 512], BF16, tag=f"gs{l}_{p}", name=f"gs{l}_{p}")
               for p in range(2)] for l in range(3)]
        xb = [st.tile([128, 2, U * BL], BF16, tag=f"xb{h}", name=f"xb{h}")
              for h in range(2)]

        for l in range(3):
            nc.vector.memset(c_st[l][:], 0.0)
            for p in range(2):
                nc.vector.memset(hT[l][p][:], 0.0)

        ew = ctx.enter_context(tc.tile_pool(name="ew", bufs=3))
        psp = ctx.enter_context(tc.tile_pool(name="psp", bufs=1, space="PSUM"))
        ps_g = [[psp.tile([128, 512], F32, tag=f"ps{l}_{p}", name=f"ps{l}_{p}")
                 for p in range(2)] for l in range(3)]
        ps_fc = psp.tile([3, BL], F32, tag="psfc")

        def mm_in_rounds(l, p, in_ap):
            """Bias + input matmul rounds for layer l, psum parity p."""
            ps = ps_g[l][p]
            for g in range(4):
                nc.tensor.matmul(
                    ps[32 * g:32 * g + 32, :], lhsT=ones_sb[:],
                    rhs=bias_sb[l][:, 512 * g:512 * g + 512],
                    start=True, stop=False, tile_position=(0, 32 * g),
                    skip_group_check=True)
            for kc in range(KIN[l]):
                lhsT = in_ap(kc)
                for g in range(4):
                    nc.tensor.matmul(
                        ps[32 * g:32 * g + 32, :], lhsT=lhsT,
                        rhs=w_sb[l][:, kc * 2048 + 512 * g:
                                    kc * 2048 + 512 * g + 512],
                        start=False, stop=False,
                        tile_position=(0, 32 * g), skip_group_check=True)

        def mm_own_rounds(l, p):
            """Recurrent (Whh) matmul rounds for layer l."""
            ps = ps_g[l][p]
            kin = KIN[l]
            for kc in range(4):
                lhsT = hT_ap(l, 1 - p, kc)
                for g in range(4):
                    nc.tensor.matmul(
                        ps[32 * g:32 * g + 32, :], lhsT=lhsT,
                        rhs=w_sb[l][:, (kin + kc) * 2048 + 512 * g:
                                    (kin + kc) * 2048 + 512 * g + 512],
                        start=False, stop=(kc == 3),
                        tile_position=(0, 32 * g), skip_group_check=True)

        def mm_rounds(l, p, in_ap):
            mm_in_rounds(l, p, in_ap)
            mm_own_rounds(l, p)

        def evac(l, p):
            """psum -> sigmoid/tanh evacuation for layer l (ACT only)."""
            ps = ps_g[l][p]
            g_ = gs[l][p]
            nc.scalar.activation(g_[:, 0:384], ps[:, 0:384], AF.Sigmoid)
            nc.scalar.activation(g_[:, 384:512], ps[:, 384:512], AF.Tanh)

        def cph(l, p):
            """cell update + h + transpose for layer l."""
            g_ = gs[l][p]
            t2 = ew.tile([128, 128], F32, tag="t2")
            nc.vector.tensor_mul(t2[:], g_[:, 128:256], c_st[l][:])
            t1 = ew.tile([128, 128], F32, tag="t1")
            nc.vector.tensor_mul(t1[:], g_[:, 0:128], g_[:, 384:512])
            nc.vector.tensor_add(c_st[l][:], t1[:], t2[:])
            th = ew.tile([128, 128], F32, tag="th")
            nc.scalar.activation(th[:], c_st[l][:], AF.Tanh)
            nc.vector.tensor_mul(h_sb[l][p][:], g_[:, 256:384], th[:])
            nc.sync.dma_start_transpose(hT[l][p][:], h_sb[l][p][:])

        def tail(l, p):
            evac(l, p)
            cph(l, p)

        def super_step(u_par, x_ap):
            """One super-step: layer l does its step; x_ap for layer 0.
            All input rounds first, then the recurrence-critical own
            rounds, maximizing slack for the h-transpose chain."""
            p = u_par
            mm_rounds(0, p, x_ap)
            mm_rounds(1, p, lambda kc: hT_ap(0, 1 - p, kc))
            mm_rounds(2, p, lambda kc: hT_ap(1, 1 - p, kc))
            evac(0, p)
            evac(1, p)
            cph(0, p)
            evac(2, p)
            cph(1, p)
            cph(2, p)

        # ---- peeled head: u=0 (l0 t=0), u=1 (l0 t=1, l1 t=0) ----
        def xboot_ap(t):
            return lambda kc: xboot[:, kc, t * BL:(t + 1) * BL]

        mm_rounds(0, 0, xboot_ap(0))
        tail(0, 0)
        mm_rounds(0, 1, xboot_ap(1))
        mm_rounds(1, 1, lambda kc: hT_ap(0, 0, kc))
        tail(0, 1)
        tail(1, 1)

        # preload x: buf0 <- t [2,7), buf1 <- t [7,12)
        nc.sync.dma_start(xb[0][:], xt.ap()[:, :, 2 * BL:(2 + U) * BL])
        nc.sync.dma_start(xb[1][:], xt.ap()[:, :, (2 + U) * BL:(2 + 2 * U) * BL])

        # ---- steady loop: iteration iv covers u = 2+10iv .. 11+10iv ----
        def body(iv):
            for j in range(2 * U):
                half = j // U
                jj = j % U

                def x_ap(kc, half=half, jj=jj):
                    return xb[half][:, kc, jj * BL:(jj + 1) * BL]
                super_step(j & 1, x_ap)
                if j == U - 1:   # refill buf0 for next iteration
                    nc.scalar.dma_start(
                        xb[0][:],
                        xt.ap()[:, :, bass.ds((iv * 2 * U + 2 + 2 * U) * BL,
                                              U * BL)])
                if j == 2 * U - 1:  # refill buf1 for next iteration
                    nc.scalar.dma_start(
                        xb[1][:],
                        xt.ap()[:, :, bass.ds((iv * 2 * U + 2 + 3 * U) * BL,
                                              U * BL)])

        with tc.For_i(0, NITER, 1,
                      hint_engines=(mybir.EngineType.PE,
                                    mybir.EngineType.Activation,
                                    mybir.EngineType.DVE)) as iv:
            body(iv)

        # ---- peeled tail: u=512 (l1 t=511, l2 t=510), u=513 (l2 t=511) ----
        # parity of u=512 is 0, u=513 is 1
        mm_rounds(1, 0, lambda kc: hT_ap(0, 1, kc))
        mm_rounds(2, 0, lambda kc: hT_ap(1, 1, kc))
        tail(1, 0)
        tail(2, 0)
        mm_rounds(2, 1, lambda kc: hT_ap(1, 0, kc))
        tail(2, 1)

        # ---- head: out.T[3, BL] = fcW @ h2(511) + fcB ----
        for kc in range(4):
            nc.tensor.matmul(
                ps_fc[:], lhsT=fcw_sb[:, kc * 3:kc * 3 + 3],
                rhs=hT_ap(2, 1, kc),
                start=(kc == 0), stop=(kc == 3))
        ob = ew.tile([3, BL], F32, tag="ob")
        nc.scalar.activation(ob[:], ps_fc[:], AF.Identity, bias=fcb_sb[:])
        nc.sync.dma_start(out_d.ap(), ob[:])

    nc.compile()
    return nc


def _prep(inputs):
    """Host-side layout prep. Returns per-core in_maps."""
    bf = ml_dtypes.bfloat16
    x = np.asarray(inputs["x"], np.float32)

    def stream_pack(w):
        """[2048, K] -> [128, (K/128)*2048] with col order (kc, g, q, jj):
        out[p, kc*2048 + g*512 + q*128 + jj] = w[512q+128g+jj, 128kc+p]."""
        K = w.shape[1]
        kcs = K // 128
        # w4[q, g, jj, kc, p]; reorder gates to (i,f,o,g) so one sigmoid
        # instruction covers cols 0:384
        w4 = w.reshape(4, 4, 128, kcs, 128)[[0, 1, 3, 2]]
        # -> [p, kc, g, q, jj]
        return np.ascontiguousarray(
            w4.transpose(4, 3, 1, 0, 2).reshape(128, kcs * 2048)).astype(bf)

    shared = {}
    for l in range(3):
        wih = np.asarray(inputs[f"Wih{l}"], np.float32)
        whh = np.asarray(inputs[f"Whh{l}"], np.float32)
        shared[f"w{l}"] = np.concatenate(
            [stream_pack(wih), stream_pack(whh)], axis=1)
        bl_ = (np.asarray(inputs[f"bih{l}"], np.float32)
               + np.asarray(inputs[f"bhh{l}"], np.float32))
        # bias[0, g*512 + q*128 + jj] = bl_[512q'+128g+jj], q' in (i,f,o,g)
        shared[f"b{l}"] = np.ascontiguousarray(
            bl_.reshape(4, 4, 128)[[0, 1, 3, 2]].transpose(1, 0, 2)
            .reshape(1, 2048)).astype(bf)
    shared["fcw"] = np.ascontiguousarray(
        np.asarray(inputs["fcW"], np.float32).T.reshape(4, 128, 3)
        .transpose(1, 0, 2).reshape(128, 12)).astype(bf)
    shared["fcb"] = np.asarray(inputs["fcB"], np.float32).reshape(3, 1)

    in_maps = []
    for c in range(NCORES):
        xc = x[c * BL:(c + 1) * BL]                       # [32, 512, 256]
        xp = xc.transpose(2, 1, 0).reshape(2, 128, T * BL)  # [2,128,16384]
        xp = np.ascontiguousarray(xp.transpose(1, 0, 2))    # [128,2,16384]
        xp = np.concatenate(
            [xp, np.zeros((128, 2, SLACK), np.float32)], axis=2).astype(bf)
        in_maps.append({"x_t": xp, **shared})
    return in_maps


_NC_CACHE = None


def kernel(**inputs):
    global _NC_CACHE
    if _NC_CACHE is None:
        _NC_CACHE = _build()
    nc = _NC_CACHE
    in_maps = _prep(inputs)
    res = run_bass_kernel_spmd(nc, in_maps, core_ids=list(range(NCORES)))
    out = np.empty((B, O), np.float32)
    for c in range(NCORES):
        out[c * BL:(c + 1) * BL] = res.results[c]["out"].T
    return out


# revision 17
# speedup vs baseline: 1.1334x; 1.1334x over previous
"""3-layer LSTM (B=256,T=512,I=256,H=512) + linear head on 8 NeuronCores.

Strategy: data-parallel over batch (32/core). Per step the gate matmul is
computed h-STATIONARY: the tiny h_t.T chunk ([128,32]) is the PE stationary
operand and the *weights* stream through the array as the moving operand,
split across the 4 PE column groups (tile_position=(0,32g)) so four N=512
weight streams run concurrently (~216ns per round of 2048 cols). This
removes the per-step LDWEIGHTS wall (8192 weight cols/step through the
slow load path) that bounds the weights-stationary form.

PSUM layout [32*slice+b, 128*q+jj] = gates.T: all 4 gates of a hidden unit
live on the same partition, so the sigmoid/tanh + cell update run with all
128 DVE/ACT lanes. h_t is transposed back to stationary form each step by
one [128,128] bf16 DMA-crossbar transpose (~1.2us, off the PE).

The 3 layers run as a wavefront (layer l computes step u-l in super-step u)
so the PE always has ~25 dense matmul rounds per super-step and the HAM
clock gate stays at 2.4GHz. h never leaves SBUF; biases enter via a K=1
ones-row matmul that also start=True-clears each psum accumulation group.
"""

import numpy as np
import ml_dtypes
from contextlib import ExitStack

import concourse.bass as bass
import concourse.bacc as bacc
import concourse.tile as tile
from concourse import mybir
from concourse.bass_utils import run_bass_kernel_spmd

BF16 = mybir.dt.bfloat16
F32 = mybir.dt.float32
AF = mybir.ActivationFunctionType

B, T, I, H, O = 256, 512, 256, 512, 3
NCORES = 8
BL = B // NCORES          # 32 batch rows per core
U = 255                   # super-steps per half-body (x chunk granularity)
NITER = 1                 # steady loop iterations; covers u in [2, 512)
SLACK = 4 * U * 32        # x prefetch overrun slack (cols)

# per-layer input K chunks (x for l0: 256 = 2 chunks; h for l1/l2: 4 chunks)
KIN = [2, 4, 4]


def _build():
    nc = bacc.Bacc("TRN2", target_bir_lowering=False, debug=False,
                   num_devices=NCORES)

    xt = nc.dram_tensor("x_t", (128, 2, T * BL + SLACK), BF16,
                        kind="ExternalInput")
    # packed weight streams: [input chunks | whh chunks], each chunk 2048 cols
    wpk = [nc.dram_tensor(f"w{l}", (128, (KIN[l] + 4) * 2048), BF16,
                          kind="ExternalInput") for l in range(3)]
    bias_d = [nc.dram_tensor(f"b{l}", (1, 2048), BF16, kind="ExternalInput")
              for l in range(3)]
    fcw_d = nc.dram_tensor("fcw", (128, 12), BF16, kind="ExternalInput")
    fcb_d = nc.dram_tensor("fcb", (3, 1), F32, kind="ExternalInput")
    out_d = nc.dram_tensor("out", (3, BL), F32, kind="ExternalOutput")

    with tile.TileContext(nc) as tc, ExitStack() as ctx:
        const = ctx.enter_context(tc.tile_pool(name="const", bufs=1))
        w_sb = [const.tile([128, (KIN[l] + 4) * 2048], BF16, tag=f"w{l}",
                           name=f"w_sb{l}") for l in range(3)]
        bias_sb = [const.tile([1, 2048], BF16, tag=f"b{l}", name=f"bias_sb{l}")
                   for l in range(3)]
        ones_sb = const.tile([1, BL], BF16, tag="ones")
        fcw_sb = const.tile([128, 12], BF16, tag="fcw")
        fcb_sb = const.tile([3, 1], F32, tag="fcb")
        xboot = const.tile([128, 2, 2 * BL], BF16, tag="xboot")
        for l in range(3):
            nc.sync.dma_start(w_sb[l][:], wpk[l].ap())
            nc.sync.dma_start(bias_sb[l][:], bias_d[l].ap())
        nc.sync.dma_start(fcw_sb[:], fcw_d.ap())
        nc.sync.dma_start(fcb_sb[:], fcb_d.ap())
        nc.sync.dma_start(xboot[:], xt.ap()[:, :, 0:2 * BL])
        nc.vector.memset(ones_sb[:], 1.0)

        st = ctx.enter_context(tc.tile_pool(name="st", bufs=1))
        # persistent state
        c_st = [st.tile([128, 128], F32, tag=f"c{l}", name=f"c_st{l}")
                for l in range(3)]
        hT = [[st.tile([128, 128], BF16, tag=f"hT{l}_{p}", name=f"hT{l}_{p}")
               for p in range(2)] for l in range(3)]

        def hT_ap(l, p, kc):
            return hT[l][p][:, 32 * kc:32 * kc + 32]
        h_sb = [[st.tile([128, 128], BF16, tag=f"h{l}_{p}", name=f"h_sb{l}_{p}")
                 for p in range(2)] for l in range(3)]
        gs = [[st.tile([128, 512], F32, tag=f"gs{l}_{p}", name=f"gs{l}_{p}")
               for p in range(2)] for l in range(3)]
        xb = [st.tile([128, 2, U * BL], BF16, tag=f"xb{h}", name=f"xb{h}")
              for h in range(2)]

        for l in range(3):
            nc.vector.memset(c_st[l][:], 0.0)
            for p in range(2):
                nc.vector.memset(hT[l][p][:], 0.0)

        ew = ctx.enter_context(tc.tile_pool(name="ew", bufs=3))
        psp = ctx.enter_context(tc.tile_pool(name="psp", bufs=1, space="PSUM"))
        ps_g = [[psp.tile([128, 512], F32, tag=f"ps{l}_{p}", name=f"ps{l}_{p}")
                 for p in range(2)] for l in range(3)]
        ps_fc = psp.tile([3, BL], F32, tag="psfc")

        def mm_in_rounds(l, p, in_ap):
            """Bias + input matmul rounds for layer l, psum parity p."""
            ps = ps_g[l][p]
            for g in range(4):
                nc.tensor.matmul(
                    ps[32 * g:32 * g + 32, :], lhsT=ones_sb[:],
                    rhs=bias_sb[l][:, 512 * g:512 * g + 512],
                    start=True, stop=False, tile_position=(0, 32 * g),
                    skip_group_check=True)
            for kc in range(KIN[l]):
                lhsT = in_ap(kc)
                for g in range(4):
                    nc.tensor.matmul(
                        ps[32 * g:32 * g + 32, :], lhsT=lhsT,
                        rhs=w_sb[l][:, kc * 2048 + 512 * g:
                                    kc * 2048 + 512 * g + 512],
                        start=False, stop=False,
                        tile_position=(0, 32 * g), skip_group_check=True)

        def mm_own_rounds(l, p):
            """Recurrent (Whh) matmul rounds for layer l."""
            ps = ps_g[l][p]
            kin = KIN[l]
            for kc in range(4):
                lhsT = hT_ap(l, 1 - p, kc)
                for g in range(4):
                    nc.tensor.matmul(
                        ps[32 * g:32 * g + 32, :], lhsT=lhsT,
                        rhs=w_sb[l][:, (kin + kc) * 2048 + 512 * g:
                                    (kin + kc) * 2048 + 512 * g + 512],
                        start=False, stop=(kc == 3),
                        tile_position=(0, 32 * g), skip_group_check=True)

        def mm_rounds(l, p, in_ap):
            mm_in_rounds(l, p, in_ap)
            mm_own_rounds(l, p)

        def evac(l, p):
            """psum -> sigmoid/tanh evacuation for layer l (ACT only)."""
            ps = ps_g[l][p]
            g_ = gs[l][p]
            nc.scalar.activation(g_[:, 0:384], ps[:, 0:384], AF.Sigmoid)
            nc.scalar.activation(g_[:, 384:512], ps[:, 384:512], AF.Tanh)

        def cph(l, p):
            """cell update + h + transpose for layer l."""
            g_ = gs[l][p]
            t2 = ew.tile([128, 128], F32, tag="t2")
            nc.vector.tensor_mul(t2[:], g_[:, 128:256], c_st[l][:])
            t1 = ew.tile([128, 128], F32, tag="t1")
            nc.vector.tensor_mul(t1[:], g_[:, 0:128], g_[:, 384:512])
            nc.vector.tensor_add(c_st[l][:], t1[:], t2[:])
            th = ew.tile([128, 128], F32, tag="th")
            nc.scalar.activation(th[:], c_st[l][:], AF.Tanh)
            nc.vector.tensor_mul(h_sb[l][p][:], g_[:, 256:384], th[:])
            nc.sync.dma_start_transpose(hT[l][p][:], h_sb[l][p][:])

        def tail(l, p):
            evac(l, p)
            cph(l, p)

        def super_step(u_par, x_ap):
            """One super-step: layer l does its step; x_ap for layer 0.
            All input rounds first, then the recurrence-critical own
            rounds, maximizing slack for the h-transpose chain."""
            p = u_par
            mm_rounds(0, p, x_ap)
            mm_rounds(1, p, lambda kc: hT_ap(0, 1 - p, kc))
            mm_rounds(2, p, lambda kc: hT_ap(1, 1 - p, kc))
            evac(0, p)
            evac(1, p)
            cph(0, p)
            evac(2, p)
            cph(1, p)
            cph(2, p)

        # ---- peeled head: u=0 (l0 t=0), u=1 (l0 t=1, l1 t=0) ----
        def xboot_ap(t):
            return lambda kc: xboot[:, kc, t * BL:(t + 1) * BL]

        mm_rounds(0, 0, xboot_ap(0))
        tail(0, 0)
        mm_rounds(0, 1, xboot_ap(1))
        mm_rounds(1, 1, lambda kc: hT_ap(0, 0, kc))
        tail(0, 1)
        tail(1, 1)

        # preload x: buf0 <- t [2,7), buf1 <- t [7,12)
        nc.sync.dma_start(xb[0][:], xt.ap()[:, :, 2 * BL:(2 + U) * BL])
        nc.sync.dma_start(xb[1][:], xt.ap()[:, :, (2 + U) * BL:(2 + 2 * U) * BL])

        # ---- steady loop: iteration iv covers u = 2+10iv .. 11+10iv ----
        def body(iv):
            for j in range(2 * U):
                half = j // U
                jj = j % U

                def x_ap(kc, half=half, jj=jj):
                    return xb[half][:, kc, jj * BL:(jj + 1) * BL]
                super_step(j & 1, x_ap)
                if j == U - 1:   # refill buf0 for next iteration
                    nc.scalar.dma_start(
                        xb[0][:],
                        xt.ap()[:, :, bass.ds((iv * 2 * U + 2 + 2 * U) * BL,
                                              U * BL)])
                if j == 2 * U - 1:  # refill buf1 for next iteration
                    nc.scalar.dma_start(
                        xb[1][:],
                        xt.ap()[:, :, bass.ds((iv * 2 * U + 2 + 3 * U) * BL,
                                              U * BL)])

        with tc.For_i(0, NITER, 1,
                      hint_engines=(mybir.EngineType.PE,
                                    mybir.EngineType.Activation,
                                    mybir.EngineType.DVE)) as iv:
            body(iv)

        # ---- peeled tail: u=512 (l1 t=511, l2 t=510), u=513 (l2 t=511) ----
        # parity of u=512 is 0, u=513 is 1
        mm_rounds(1, 0, lambda kc: hT_ap(0, 1, kc))
        mm_rounds(2, 0, lambda kc: hT_ap(1, 1, kc))
        tail(1, 0)
        tail(2, 0)
        mm_rounds(2, 1, lambda kc: hT_ap(1, 0, kc))
        tail(2, 1)

        # ---- head: out.T[3, BL] = fcW @ h2(511) + fcB ----
        for kc in range(4):
            nc.tensor.matmul(
                ps_fc[:], lhsT=fcw_sb[:, kc * 3:kc * 3 + 3],
                rhs=hT_ap(2, 1, kc),
                start=(kc == 0), stop=(kc == 3))
        ob = ew.tile([3, BL], F32, tag="ob")
        nc.scalar.activation(ob[:], ps_fc[:], AF.Identity, bias=fcb_sb[:])
        nc.sync.dma_start(out_d.ap(), ob[:])

    nc.compile()
    return nc


def _prep(inputs):
    """Host-side layout prep. Returns per-core in_maps."""
    bf = ml_dtypes.bfloat16
    x = np.asarray(inputs["x"], np.float32)

    def stream_pack(w):
        """[2048, K] -> [128, (K/128)*2048] with col order (kc, g, q, jj):
        out[p, kc*2048 + g*512 + q*128 + jj] = w[512q+128g+jj, 128kc+p]."""
        K = w.shape[1]
        kcs = K // 128
        # w4[q, g, jj, kc, p]; reorder gates to (i,f,o,g) so one sigmoid
        # instruction covers cols 0:384
        w4 = w.reshape(4, 4, 128, kcs, 128)[[0, 1, 3, 2]]
        # -> [p, kc, g, q, jj]
        return np.ascontiguousarray(
            w4.transpose(4, 3, 1, 0, 2).reshape(128, kcs * 2048)).astype(bf)

    shared = {}
    for l in range(3):
        wih = np.asarray(inputs[f"Wih{l}"], np.float32)
        whh = np.asarray(inputs[f"Whh{l}"], np.float32)
        shared[f"w{l}"] = np.concatenate(
            [stream_pack(wih), stream_pack(whh)], axis=1)
        bl_ = (np.asarray(inputs[f"bih{l}"], np.float32)
               + np.asarray(inputs[f"bhh{l}"], np.float32))
        # bias[0, g*512 + q*128 + jj] = bl_[512q'+128g+jj], q' in (i,f,o,g)
        shared[f"b{l}"] = np.ascontiguousarray(
            bl_.reshape(4, 4, 128)[[0, 1, 3, 2]].transpose(1, 0, 2)
            .reshape(1, 2048)).astype(bf)
    shared["fcw"] = np.ascontiguousarray(
        np.asarray(inputs["fcW"], np.float32).T.reshape(4, 128, 3)
        .transpose(1, 0, 2).reshape(128, 12)).astype(bf)
    shared["fcb"] = np.asarray(inputs["fcB"], np.float32).reshape(3, 1)

    in_maps = []
    for c in range(NCORES):
        xc = x[c * BL:(c + 1) * BL]                       # [32, 512, 256]
        xp = xc.transpose(2, 1, 0).reshape(2, 128, T * BL)  # [2,128,16384]
        xp = np.ascontiguousarray(xp.transpose(1, 0, 2))    # [128,2,16384]
        xp = np.concatenate(
            [xp, np.zeros((128, 2, SLACK), np.float32)], axis=2).astype(bf)
        in_maps.append({"x_t": xp, **shared})
    return in_maps


_NC_CACHE = None


def kernel(**inputs):
    global _NC_CACHE
    if _NC_CACHE is None:
        _NC_CACHE = _build()
    nc = _NC_CACHE
    in_maps = _prep(inputs)
    res = run_bass_kernel_spmd(nc, in_maps, core_ids=list(range(NCORES)))
    out = np.empty((B, O), np.float32)
    for c in range(NCORES):
        out[c * BL:(c + 1) * BL] = res.results[c]["out"].T
    return out


# revision 19
# speedup vs baseline: 1.1455x; 1.0106x over previous
"""3-layer LSTM (B=256,T=512,I=256,H=512) + linear head on 8 NeuronCores.

Strategy: data-parallel over batch (32/core). Per step the gate matmul is
computed h-STATIONARY: the tiny h_t.T chunk ([128,32]) is the PE stationary
operand and the *weights* stream through the array as the moving operand,
split across the 4 PE column groups (tile_position=(0,32g)) so four N=512
weight streams run concurrently (~216ns per round of 2048 cols). This
removes the per-step LDWEIGHTS wall (8192 weight cols/step through the
slow load path) that bounds the weights-stationary form.

PSUM layout [32*slice+b, 128*q+jj] = gates.T: all 4 gates of a hidden unit
live on the same partition, so the sigmoid/tanh + cell update run with all
128 DVE/ACT lanes. h_t is transposed back to stationary form each step by
one [128,128] bf16 DMA-crossbar transpose (~1.2us, off the PE).

The 3 layers run as a wavefront (layer l computes step u-l in super-step u)
so the PE always has ~25 dense matmul rounds per super-step and the HAM
clock gate stays at 2.4GHz. h never leaves SBUF; biases enter via a K=1
ones-row matmul that also start=True-clears each psum accumulation group.
"""

import numpy as np
import ml_dtypes
from contextlib import ExitStack

import concourse.bass as bass
import concourse.bacc as bacc
import concourse.tile as tile
from concourse import mybir
from concourse.bass_utils import run_bass_kernel_spmd

BF16 = mybir.dt.bfloat16
F32 = mybir.dt.float32
AF = mybir.ActivationFunctionType

B, T, I, H, O = 256, 512, 256, 512, 3
NCORES = 8
BL = B // NCORES          # 32 batch rows per core
U = 255                   # super-steps per half-body (x chunk granularity)
NITER = 1                 # steady loop iterations; covers u in [2, 512)
SLACK = 4 * U * 32        # x prefetch overrun slack (cols)

# per-layer input K chunks (x for l0: 256 = 2 chunks; h for l1/l2: 4 chunks)
KIN = [2, 4, 4]


def _build():
    nc = bacc.Bacc("TRN2", target_bir_lowering=False, debug=False,
                   num_devices=NCORES)

    xt = nc.dram_tensor("x_t", (128, 2, T * BL + SLACK), BF16,
                        kind="ExternalInput")
    # packed weight streams: [input chunks | whh chunks], each chunk 2048 cols
    wpk = [nc.dram_tensor(f"w{l}", (128, (KIN[l] + 4) * 2048), BF16,
                          kind="ExternalInput") for l in range(3)]
    bias_d = [nc.dram_tensor(f"b{l}", (1, 2048), BF16, kind="ExternalInput")
              for l in range(3)]
    fcw_d = nc.dram_tensor("fcw", (128, 12), BF16, kind="ExternalInput")
    fcb_d = nc.dram_tensor("fcb", (3, 1), F32, kind="ExternalInput")
    out_d = nc.dram_tensor("out", (3, BL), F32, kind="ExternalOutput")

    with tile.TileContext(nc) as tc, ExitStack() as ctx:
        const = ctx.enter_context(tc.tile_pool(name="const", bufs=1))
        w_sb = [const.tile([128, (KIN[l] + 4) * 2048], BF16, tag=f"w{l}",
                           name=f"w_sb{l}") for l in range(3)]
        bias_sb = [const.tile([1, 2048], BF16, tag=f"b{l}", name=f"bias_sb{l}")
                   for l in range(3)]
        ones_sb = const.tile([1, BL], BF16, tag="ones")
        fcw_sb = const.tile([128, 12], BF16, tag="fcw")
        fcb_sb = const.tile([3, 1], F32, tag="fcb")
        xboot = const.tile([128, 2, 2 * BL], BF16, tag="xboot")
        for l in range(3):
            nc.sync.dma_start(w_sb[l][:], wpk[l].ap())
            nc.sync.dma_start(bias_sb[l][:], bias_d[l].ap())
        nc.sync.dma_start(fcw_sb[:], fcw_d.ap())
        nc.sync.dma_start(fcb_sb[:], fcb_d.ap())
        nc.sync.dma_start(xboot[:], xt.ap()[:, :, 0:2 * BL])
        nc.vector.memset(ones_sb[:], 1.0)

        st = ctx.enter_context(tc.tile_pool(name="st", bufs=1))
        # persistent state
        c_st = [st.tile([128, 128], F32, tag=f"c{l}", name=f"c_st{l}")
                for l in range(3)]
        hT = [[st.tile([128, 128], BF16, tag=f"hT{l}_{p}", name=f"hT{l}_{p}")
               for p in range(2)] for l in range(3)]

        def hT_ap(l, p, kc):
            return hT[l][p][:, 32 * kc:32 * kc + 32]
        h_sb = [[st.tile([128, 128], BF16, tag=f"h{l}_{p}", name=f"h_sb{l}_{p}")
                 for p in range(2)] for l in range(3)]
        gs = [[st.tile([128, 512], F32, tag=f"gs{l}_{p}", name=f"gs{l}_{p}")
               for p in range(2)] for l in range(3)]
        xb = [st.tile([128, 2, U * BL], BF16, tag=f"xb{h}", name=f"xb{h}")
              for h in range(2)]

        for l in range(3):
            nc.vector.memset(c_st[l][:], 0.0)
            for p in range(2):
                nc.vector.memset(hT[l][p][:], 0.0)

        ew = ctx.enter_context(tc.tile_pool(name="ew", bufs=3))
        psp = ctx.enter_context(tc.tile_pool(name="psp", bufs=1, space="PSUM"))
        ps_g = [[psp.tile([128, 512], F32, tag=f"ps{l}_{p}", name=f"ps{l}_{p}")
                 for p in range(2)] for l in range(3)]
        ps_fc = psp.tile([3, BL], F32, tag="psfc")

        def mm_in_rounds(l, p, in_ap):
            """Bias + input matmul rounds for layer l, psum parity p."""
            ps = ps_g[l][p]
            for g in range(4):
                nc.tensor.matmul(
                    ps[32 * g:32 * g + 32, :], lhsT=ones_sb[:],
                    rhs=bias_sb[l][:, 512 * g:512 * g + 512],
                    start=True, stop=False, tile_position=(0, 32 * g),
                    skip_group_check=True)
            for kc in range(KIN[l]):
                lhsT = in_ap(kc)
                for g in range(4):
                    nc.tensor.matmul(
                        ps[32 * g:32 * g + 32, :], lhsT=lhsT,
                        rhs=w_sb[l][:, kc * 2048 + 512 * g:
                                    kc * 2048 + 512 * g + 512],
                        start=False, stop=False,
                        tile_position=(0, 32 * g), skip_group_check=True)

        def mm_own_rounds(l, p):
            """Recurrent (Whh) matmul rounds for layer l."""
            ps = ps_g[l][p]
            kin = KIN[l]
            for kc in range(4):
                lhsT = hT_ap(l, 1 - p, kc)
                for g in range(4):
                    nc.tensor.matmul(
                        ps[32 * g:32 * g + 32, :], lhsT=lhsT,
                        rhs=w_sb[l][:, (kin + kc) * 2048 + 512 * g:
                                    (kin + kc) * 2048 + 512 * g + 512],
                        start=False, stop=(kc == 3),
                        tile_position=(0, 32 * g), skip_group_check=True)

        def mm_rounds(l, p, in_ap):
            mm_in_rounds(l, p, in_ap)
            mm_own_rounds(l, p)

        def evac(l, p):
            """psum -> sigmoid/tanh evacuation for layer l (ACT only)."""
            ps = ps_g[l][p]
            g_ = gs[l][p]
            nc.scalar.activation(g_[:, 0:384], ps[:, 0:384], AF.Sigmoid)
            nc.scalar.activation(g_[:, 384:512], ps[:, 384:512], AF.Tanh)

        def cph(l, p):
            """cell update + h + transpose for layer l."""
            g_ = gs[l][p]
            t2 = ew.tile([128, 128], F32, tag="t2")
            nc.vector.tensor_mul(t2[:], g_[:, 128:256], c_st[l][:])
            t1 = ew.tile([128, 128], F32, tag="t1")
            nc.vector.tensor_mul(t1[:], g_[:, 0:128], g_[:, 384:512])
            nc.vector.tensor_add(c_st[l][:], t1[:], t2[:])
            th = ew.tile([128, 128], F32, tag="th")
            nc.scalar.activation(th[:], c_st[l][:], AF.Tanh)
            nc.vector.tensor_mul(h_sb[l][p][:], g_[:, 256:384], th[:])
            nc.sync.dma_start_transpose(hT[l][p][:], h_sb[l][p][:])

        def tail(l, p):
            evac(l, p)
            cph(l, p)

        def super_step(u_par, x_ap):
            """One super-step: layer l does its step; x_ap for layer 0.
            All input rounds first, then the recurrence-critical own
            rounds, maximizing slack for the h-transpose chain."""
            p = u_par
            mm_rounds(0, p, x_ap)
            mm_rounds(1, p, lambda kc: hT_ap(0, 1 - p, kc))
            mm_rounds(2, p, lambda kc: hT_ap(1, 1 - p, kc))
            evac(0, p)
            evac(1, p)
            cph(0, p)
            evac(2, p)
            cph(1, p)
            cph(2, p)

        # ---- peeled head: u=0 (l0 t=0), u=1 (l0 t=1, l1 t=0) ----
        def xboot_ap(t):
            return lambda kc: xboot[:, kc, t * BL:(t + 1) * BL]

        mm_rounds(0, 0, xboot_ap(0))
        tail(0, 0)
        mm_rounds(0, 1, xboot_ap(1))
        mm_rounds(1, 1, lambda kc: hT_ap(0, 0, kc))
        tail(0, 1)
        tail(1, 1)

        # preload x: buf0 <- t [2,7), buf1 <- t [7,12)
        nc.sync.dma_start(xb[0][:], xt.ap()[:, :, 2 * BL:(2 + U) * BL])
        nc.sync.dma_start(xb[1][:], xt.ap()[:, :, (2 + U) * BL:(2 + 2 * U) * BL])

        # ---- steady loop: iteration iv covers u = 2+10iv .. 11+10iv ----
        def body(iv):
            for j in range(2 * U):
                half = j // U
                jj = j % U

                def x_ap(kc, half=half, jj=jj):
                    return xb[half][:, kc, jj * BL:(jj + 1) * BL]
                super_step(j & 1, x_ap)
                if j == U - 1:   # refill buf0 for next iteration
                    nc.scalar.dma_start(
                        xb[0][:],
                        xt.ap()[:, :, bass.ds((iv * 2 * U + 2 + 2 * U) * BL,
                                              U * BL)])
                if j == 2 * U - 1:  # refill buf1 for next iteration
                    nc.scalar.dma_start(
                        xb[1][:],
                        xt.ap()[:, :, bass.ds((iv * 2 * U + 2 + 3 * U) * BL,
                                              U * BL)])

        with tc.For_i(0, NITER, 1,
                      hint_engines=(mybir.EngineType.PE,
                                    mybir.EngineType.Activation,
                                    mybir.EngineType.DVE)) as iv:
            body(iv)

        # ---- peeled tail: u=512 (l1 t=511, l2 t=510), u=513 (l2 t=511) ----
        # parity of u=512 is 0, u=513 is 1
        mm_rounds(1, 0, lambda kc: hT_ap(0, 1, kc))
        mm_rounds(2, 0, lambda kc: hT_ap(1, 1, kc))
        tail(1, 0)
        tail(2, 0)
        mm_rounds(2, 1, lambda kc: hT_ap(1, 0, kc))
        tail(2, 1)

        # ---- head: out.T[3, BL] = fcW @ h2(511) + fcB ----
        for kc in range(4):
            nc.tensor.matmul(
                ps_fc[:], lhsT=fcw_sb[:, kc * 3:kc * 3 + 3],
                rhs=hT_ap(2, 1, kc),
                start=(kc == 0), stop=(kc == 3))
        ob = ew.tile([3, BL], F32, tag="ob")
        nc.scalar.activation(ob[:], ps_fc[:], AF.Identity, bias=fcb_sb[:])
        nc.sync.dma_start(out_d.ap(), ob[:])

    nc.compile()
    return nc


def _prep(inputs):
    """Host-side layout prep. Returns per-core in_maps."""
    bf = ml_dtypes.bfloat16
    x = np.asarray(inputs["x"], np.float32)

    def stream_pack(w):
        """[2048, K] -> [128, (K/128)*2048] with col order (kc, g, q, jj):
        out[p, kc*2048 + g*512 + q*128 + jj] = w[512q+128g+jj, 128kc+p]."""
        K = w.shape[1]
        kcs = K // 128
        # w4[q, g, jj, kc, p]; reorder gates to (i,f,o,g) so one sigmoid
        # instruction covers cols 0:384
        w4 = w.reshape(4, 4, 128, kcs, 128)[[0, 1, 3, 2]]
        # -> [p, kc, g, q, jj]
        return np.ascontiguousarray(
            w4.transpose(4, 3, 1, 0, 2).reshape(128, kcs * 2048)).astype(bf)

    shared = {}
    for l in range(3):
        wih = np.asarray(inputs[f"Wih{l}"], np.float32)
        whh = np.asarray(inputs[f"Whh{l}"], np.float32)
        shared[f"w{l}"] = np.concatenate(
            [stream_pack(wih), stream_pack(whh)], axis=1)
        bl_ = (np.asarray(inputs[f"bih{l}"], np.float32)
               + np.asarray(inputs[f"bhh{l}"], np.float32))
        # bias[0, g*512 + q*128 + jj] = bl_[512q'+128g+jj], q' in (i,f,o,g)
        shared[f"b{l}"] = np.ascontiguousarray(
            bl_.reshape(4, 4, 128)[[0, 1, 3, 2]].transpose(1, 0, 2)
            .reshape(1, 2048)).astype(bf)
    shared["fcw"] = np.ascontiguousarray(
        np.asarray(inputs["fcW"], np.float32).T.reshape(4, 128, 3)
        .transpose(1, 0, 2).reshape(128, 12)).astype(bf)
    shared["fcb"] = np.asarray(inputs["fcB"], np.float32).reshape(3, 1)

    in_maps = []
    for c in range(NCORES):
        xc = x[c * BL:(c + 1) * BL]                       # [32, 512, 256]
        xp = xc.transpose(2, 1, 0).reshape(2, 128, T * BL)  # [2,128,16384]
        xp = np.ascontiguousarray(xp.transpose(1, 0, 2))    # [128,2,16384]
        xp = np.concatenate(
            [xp, np.zeros((128, 2, SLACK), np.float32)], axis=2).astype(bf)
        in_maps.append({"x_t": xp, **shared})
    return in_maps


_NC_CACHE = None


def kernel(**inputs):
    global _NC_CACHE
    if _NC_CACHE is None:
        _NC_CACHE = _build()
    nc = _NC_CACHE
    in_maps = _prep(inputs)
    res = run_bass_kernel_spmd(nc, in_maps, core_ids=list(range(NCORES)))
    out = np.empty((B, O), np.float32)
    for c in range(NCORES):
        out[c * BL:(c + 1) * BL] = res.results[c]["out"].T
    return out
